# revision 5
# baseline (speedup 1.0000x reference)
"""Trainium2 Bass kernel for nn_Block_59433757442280 (spiking-NN local-attention block).

Sharding: data-parallel over B=8 (one batch element per NeuronCore), no collectives.
On-device layout: activations transposed [C, tok] with tok = t*1024 + n.
All GEMMs in bf16 (validated: reproduces the fp32 reference exactly on these inputs,
the LIF threshold margins downstream are structural); LIF membrane state in bf16,
softmax in fp32/bf16. The local-attention mask is folded into the sim matmul via
16 extra one-hot contraction rows.
"""

import sys

for _p in ("/opt/trn_rl_repo",):
    if _p not in sys.path:
        sys.path.insert(0, _p)

import numpy as np
import ml_dtypes

import concourse.bass as bass
import concourse.tile as tile
from concourse import mybir, bacc
from concourse.bass_utils import run_bass_kernel_spmd

F32 = mybir.dt.float32
BF16 = mybir.dt.bfloat16
AF = mybir.ActivationFunctionType
ALU = mybir.AluOpType
BF = ml_dtypes.bfloat16

# problem constants
T, B, NSEQ, C, HD = 4, 8, 1024, 768, 3072
NH, DH, W = 8, 96, 8
TOK = T * NSEQ                      # 4096 tokens per core
SCALE = float(DH) ** -0.5
NEG = -30000.0                      # mask offset (exp(scale*NEG) == 0.0 in fp32)
NCH = 256                           # phase-A/B chunk size along n
NCHUNK = NSEQ // NCH                # 4 chunks
CI6 = C // 128                      # 6 contraction tiles of 128
M24 = HD // 128                     # 24 f1 output tiles
VTH2 = 2.0                          # 2*vth for qkv/proj/mlp LIF
VTH2A = 1.0                         # 2*vth for attn lif (vth=0.5)


def _lif_head(nc, pools, psum_ap_of_t, bias_ap, dst_of_t, vth2, nt=T):
    """Emit LIF over t for one tile group.

    psum_ap_of_t(t) -> [P, n] fp32 PSUM AP of the GEMM output at step t
    bias_ap        -> [P, 1] f32 SBUF AP (per-partition bias) or None
    dst_of_t(t)    -> [P, n] bf16 SBUF AP to write spikes into
    u state: u_{t+1} = u_t * (0.5*[u_t < vth2]) + y_{t+1};  s_t = [u_t >= vth2]
    """
    upool, gpool = pools
    u_prev = None
    for t in range(nt):
        y = psum_ap_of_t(t)
        p, n = y.shape[0], y.shape[-1]
        if u_prev is None:
            u = upool.tile([p, n], BF16, name="u", tag="u")
            if bias_ap is not None:
                nc.scalar.activation(u[:], y, AF.Identity, bias=bias_ap)
            else:
                nc.vector.tensor_copy(u[:], y)
        else:
            g = gpool.tile([p, n], BF16, name="g", tag="g")
            nc.vector.tensor_scalar(g[:], u_prev[:], vth2, 0.5, ALU.is_lt, ALU.mult)
            ug = gpool.tile([p, n], BF16, name="ug", tag="ug")
            nc.vector.tensor_tensor(ug[:], u_prev[:], g[:], ALU.mult)
            if bias_ap is not None:
                yb = gpool.tile([p, n], BF16, name="yb", tag="yb")
                nc.scalar.activation(yb[:], y, AF.Identity, bias=bias_ap)
                u = upool.tile([p, n], BF16, name="u", tag="u")
                nc.vector.tensor_tensor(u[:], ug[:], yb[:], ALU.add)
            else:
                u = upool.tile([p, n], BF16, name="u", tag="u")
                nc.vector.tensor_tensor(u[:], ug[:], y, ALU.add)
        nc.vector.tensor_scalar(dst_of_t(t), u[:], vth2, None, ALU.is_ge)
        u_prev = u


def build_nc(debug=False):
    nc = bacc.Bacc(None, target_bir_lowering=False, debug=False)

    # ---- DRAM declarations (per core) ----
    xT_bf = nc.dram_tensor("xT_bf", [CI6, 128, TOK], BF16, kind="ExternalInput")
    xT_f32 = nc.dram_tensor("xT_f32", [CI6, 128, TOK], F32, kind="ExternalInput")
    wq_d = nc.dram_tensor("wq", [128, CI6 * NH * DH], BF16, kind="ExternalInput")
    wk_d = nc.dram_tensor("wk", [128, CI6 * NH * DH], BF16, kind="ExternalInput")
    wv_d = nc.dram_tensor("wv", [128, CI6 * C], BF16, kind="ExternalInput")
    wp_d = nc.dram_tensor("wp", [DH, NH * C], BF16, kind="ExternalInput")
    w1_d = nc.dram_tensor("w1", [128, CI6 * HD], BF16, kind="ExternalInput")
    w2_d = nc.dram_tensor("w2", [128, M24 * C], BF16, kind="ExternalInput")
    bq_d = nc.dram_tensor("bq", [DH, NH], F32, kind="ExternalInput")
    bk_d = nc.dram_tensor("bk", [DH, NH], F32, kind="ExternalInput")
    bvf_d = nc.dram_tensor("bvf", [128, C], BF16, kind="ExternalInput")
    bp_d = nc.dram_tensor("bp", [128, CI6], F32, kind="ExternalInput")
    b1_d = nc.dram_tensor("b1", [128, M24], F32, kind="ExternalInput")
    b2_d = nc.dram_tensor("b2", [128, CI6], F32, kind="ExternalInput")
    qext_d = nc.dram_tensor("qext_pat", [16, NCH * T], BF16, kind="ExternalInput")
    kext_d = nc.dram_tensor("kext_pat", [16, NCH * T], BF16, kind="ExternalInput")
    khp_d = nc.dram_tensor("khalo_pat", [16, T * W], BF16, kind="ExternalInput")
    khf_d = nc.dram_tensor("khalo_first", [16, T * W], BF16, kind="ExternalInput")
    id_d = nc.dram_tensor("ident", [128, 128], BF16, kind="ExternalInput")

    opT = nc.dram_tensor("opT", [CI6, 128, TOK], BF16,
                         kind="ExternalOutput" if debug else "Internal")
    outT = nc.dram_tensor("outT", [CI6, 128, TOK], F32, kind="ExternalOutput")
    if debug:
        dbg_q = nc.dram_tensor("dbg_q", [NH, DH, TOK], BF16, kind="ExternalOutput")
        dbg_k = nc.dram_tensor("dbg_k", [NH, DH, TOK], BF16, kind="ExternalOutput")
        dbg_v = nc.dram_tensor("dbg_v", [TOK, C], BF16, kind="ExternalOutput")
        dbg_oa = nc.dram_tensor("dbg_oa", [NH, DH, TOK], BF16, kind="ExternalOutput")
        dbg_h = nc.dram_tensor("dbg_h", [M24, 128, TOK], BF16, kind="ExternalOutput")

    def tok3(dram_i, c):
        """chunk AP [128, T, NCH] of dram tensor slice i at chunk c"""
        return dram_i.rearrange("p (t n) -> p t n", t=T)[:, :, c * NCH:(c + 1) * NCH]

    with tile.TileContext(nc) as tc:
        from contextlib import ExitStack
        with ExitStack() as top:
            # ======================= PHASE A =======================
            pa = top.enter_context(ExitStack())
            cpool = pa.enter_context(tc.tile_pool(name="const", bufs=1))
            # persistent attention tiles
            perspool = pa.enter_context(tc.tile_pool(name="pers", bufs=1))

            ident = cpool.tile([128, 128], BF16, name="ident", tag="ident")
            nc.sync.dma_start(ident[:], id_d[:])
            bq_sb = cpool.tile([DH, NH], F32, name="bq", tag="bq")
            nc.sync.dma_start(bq_sb[:], bq_d[:])
            bk_sb = cpool.tile([DH, NH], F32, name="bk", tag="bk")
            nc.sync.dma_start(bk_sb[:], bk_d[:])
            bvf_sb = cpool.tile([128, C], BF16, name="bvf", tag="bvf")
            nc.sync.dma_start(bvf_sb[:], bvf_d[:])
            bp_sb = cpool.tile([128, CI6], F32, name="bp", tag="bp")
            nc.sync.dma_start(bp_sb[:], bp_d[:])

            q_ext = [perspool.tile([112, T * NCH], BF16, name=f"qx{h}", tag=f"qx{h}") for h in range(NH)]
            k_ext = [perspool.tile([112, T * NCH], BF16, name=f"kx{h}", tag=f"kx{h}") for h in range(NH)]
            kh_cur = [perspool.tile([112, T, W], BF16, name=f"khc{h}", tag=f"khc{h}") for h in range(NH)]
            kh_prev = [perspool.tile([112, T, W], BF16, name=f"khp{h}", tag=f"khp{h}") for h in range(NH)]
            kh_first = [perspool.tile([112, T, W], BF16, name=f"khf{h}", tag=f"khf{h}") for h in range(NH)]
            vt_t = [[perspool.tile([128, C], BF16, name=f"vt{t}_{qh}", tag=f"vt{t}_{qh}") for qh in range(2)]
                    for t in range(T)]
            vh_cur = [perspool.tile([W, C], BF16, name=f"vhc{t}", tag=f"vhc{t}") for t in range(T)]
            vh_prev = [perspool.tile([W, C], BF16, name=f"vhp{t}", tag=f"vhp{t}") for t in range(T)]
            oa = [perspool.tile([DH, T * NCH], BF16, name=f"oa{h}", tag=f"oa{h}") for h in range(NH)]

            # init ext rows / halos
            for h in range(NH):
                nc.sync.dma_start(q_ext[h][96:112, :], qext_d[:])
                nc.sync.dma_start(k_ext[h][96:112, :], kext_d[:])
                nc.sync.dma_start(
                    kh_cur[h][96:112, :, :], khp_d.rearrange("g (t w) -> g t w", t=T))
                nc.sync.dma_start(
                    kh_prev[h][96:112, :, :], khp_d.rearrange("g (t w) -> g t w", t=T))
                nc.sync.dma_start(
                    kh_first[h][96:112, :, :], khf_d.rearrange("g (t w) -> g t w", t=T))
                nc.vector.memset(kh_prev[h][0:96, :, :], 0.0)
                nc.vector.memset(kh_first[h][0:96, :, :], 0.0)
            for t in range(T):
                nc.vector.memset(vh_prev[t][:], 0.0)

            if True:
                wpoolA = pa.enter_context(tc.tile_pool(name="wA", bufs=1))
                xpool = pa.enter_context(tc.tile_pool(name="xA", bufs=2))
                upool = pa.enter_context(tc.tile_pool(name="uA", bufs=3))
                gpool = pa.enter_context(tc.tile_pool(name="gA", bufs=2))
                apool = pa.enter_context(tc.tile_pool(name="attn", bufs=3))
                oppool = pa.enter_context(tc.tile_pool(name="op", bufs=2))
                qkv_ps = pa.enter_context(
                    tc.tile_pool(name="qkvps", bufs=2, space="PSUM"))
                sm_ps = pa.enter_context(
                    tc.tile_pool(name="smps", bufs=4, space="PSUM"))

                wq_sb = wpoolA.tile([128, CI6 * NH * DH], BF16, name="wq", tag="wq")
                nc.sync.dma_start(wq_sb[:], wq_d[:])
                wk_sb = wpoolA.tile([128, CI6 * NH * DH], BF16, name="wk", tag="wk")
                nc.sync.dma_start(wk_sb[:], wk_d[:])
                wv_sb = wpoolA.tile([128, CI6 * C], BF16, name="wv", tag="wv")
                nc.sync.dma_start(wv_sb[:], wv_d[:])
                wp_sb = wpoolA.tile([DH, NH * C], BF16, name="wp", tag="wp")
                nc.sync.dma_start(wp_sb[:], wp_d[:])

                for c in range(NCHUNK):
                    first_chunk = (c == 0)
                    # ---- load x chunk ----
                    xc = []
                    for i in range(CI6):
                        xi = xpool.tile([128, T, NCH], BF16, name=f"xc{i}", tag=f"xc{i}")
                        nc.sync.dma_start(xi[:], tok3(xT_bf[i], c))
                        xc.append(xi)

                    # ---- q, k GEMM + LIF -> q_ext/k_ext rows 0:96 ----
                    for w_sb, b_sb, ext in ((wq_sb, bq_sb, q_ext), (wk_sb, bk_sb, k_ext)):
                        for h in range(NH):
                            ps = qkv_ps.tile([DH, T, NCH], F32, name="qkvps", tag="qkvps")
                            for ci in range(CI6):
                                lhsT = w_sb[:, (ci * NH + h) * DH:(ci * NH + h + 1) * DH]
                                for hf in range(2):
                                    nc.tensor.matmul(
                                        ps[:, 2 * hf:2 * hf + 2, :], lhsT,
                                        xc[ci][:, 2 * hf:2 * hf + 2, :],
                                        start=(ci == 0), stop=(ci == CI6 - 1))
                            _lif_head(
                                nc, (upool, gpool),
                                lambda t, ps=ps: ps[:, t, :],
                                b_sb[:, h:h + 1],
                                lambda t, ext=ext, h=h: ext[h][0:96, t * NCH:(t + 1) * NCH],
                                VTH2)

                    # ---- v GEMM (x-stationary -> v.T layout) + LIF ----
                    for qh in range(2):
                        psv_of_t = []
                        for t in range(T):
                            psv = qkv_ps.tile([128, C], F32, name="qkvps", tag="qkvps")
                            for ci in range(CI6):
                                stat = xc[ci][:, t, qh * 128:(qh + 1) * 128]
                                nc.tensor.matmul(psv[:, 0:512], stat,
                                                 wv_sb[:, ci * C:ci * C + 512],
                                                 start=(ci == 0), stop=(ci == CI6 - 1))
                                nc.tensor.matmul(psv[:, 512:C], stat,
                                                 wv_sb[:, ci * C + 512:(ci + 1) * C],
                                                 start=(ci == 0), stop=(ci == CI6 - 1))
                            psv_of_t.append(psv)
                        # LIF over t in v.T layout with full-width bias
                        u_prev = None
                        for t in range(T):
                            y = psv_of_t[t]
                            if u_prev is None:
                                u = upool.tile([128, C], BF16, name="uv", tag="uv")
                                nc.vector.tensor_tensor(u[:], y[:], bvf_sb[:], ALU.add)
                            else:
                                g = gpool.tile([128, C], BF16, name="gv", tag="gv")
                                nc.vector.tensor_scalar(g[:], u_prev[:], VTH2, 0.5,
                                                        ALU.is_lt, ALU.mult)
                                ug = gpool.tile([128, C], BF16, name="ugv", tag="ugv")
                                nc.vector.tensor_tensor(ug[:], u_prev[:], g[:], ALU.mult)
                                ub = gpool.tile([128, C], BF16, name="ubv", tag="ubv")
                                nc.vector.tensor_tensor(ub[:], ug[:], bvf_sb[:], ALU.add)
                                u = upool.tile([128, C], BF16, name="uv", tag="uv")
                                nc.vector.tensor_tensor(u[:], ub[:], y[:], ALU.add)
                            nc.vector.tensor_scalar(vt_t[t][qh][:], u[:], VTH2, None,
                                                    ALU.is_ge)
                            u_prev = u

                    # halo captures needed within this chunk (qb=1 halos)
                    for h in range(NH):
                        nc.vector.tensor_copy(
                            kh_cur[h][0:96, :, :],
                            k_ext[h][0:96, :].rearrange("p (t n) -> p t n", t=T)
                            [:, :, 120:128])
                    for t in range(T):
                        nc.sync.dma_start(vh_cur[t][:], vt_t[t][0][120:128, :])

                    # ---- attention + attn-LIF -> oa ----
                    for h in range(NH):
                        u_o = None
                        for t in range(T):
                            o_ps = sm_ps.tile([DH, NCH], F32, name="attnsm", tag="attnsm")
                            for qb in range(2):
                                qc = t * NCH + qb * 128
                                sim = sm_ps.tile([128, 136], F32, name="attnsm", tag="attnsm")
                                nc.tensor.matmul(
                                    sim[:, 0:128], q_ext[h][0:112, qc:qc + 128],
                                    k_ext[h][0:112, qc:qc + 128], start=True, stop=True)
                                halo = (kh_first[h] if (first_chunk and qb == 0)
                                        else kh_prev[h] if qb == 0 else kh_cur[h])
                                nc.tensor.matmul(
                                    sim[:, 128:136], q_ext[h][0:112, qc:qc + 128],
                                    halo[0:112, t, :], start=True, stop=True)
                                attn = apool.tile([128, 136], BF16, name="attn", tag="attn")
                                rs = apool.tile([128, 1], F32, name="rs", tag="rs")
                                nc.scalar.activation(attn[:], sim[:], AF.Exp,
                                                     scale=SCALE, accum_out=rs[:])
                                rc = apool.tile([128, 1], F32, name="rc", tag="rc")
                                nc.vector.reciprocal(rc[:], rs[:])
                                attn_n = apool.tile([128, 136], BF16, name="attnn", tag="attnn")
                                nc.vector.tensor_scalar(attn_n[:], attn[:], rc[:], None,
                                                        ALU.mult)
                                tpm = sm_ps.tile([128, 128], BF16, name="attnsm", tag="attnsm")
                                nc.tensor.transpose(tpm[:], attn_n[:, 0:128], ident[:])
                                tph = sm_ps.tile([8, 128], BF16, name="attnsm", tag="attnsm")
                                nc.tensor.transpose(tph[:], attn_n[:, 128:136], ident[:])
                                am = apool.tile([128, 128], BF16, name="am", tag="am")
                                nc.scalar.copy(am[:], tpm[:])
                                ah = apool.tile([8, 128], BF16, name="ah", tag="ah")
                                nc.vector.tensor_copy(ah[:], tph[:])
                                vmain = vt_t[t][qb][:, h * DH:(h + 1) * DH]
                                vhalo = (vh_prev[t] if qb == 0 else vh_cur[t])
                                nc.tensor.matmul(o_ps[:, qb * 128:(qb + 1) * 128],
                                                 vmain, am[:], start=True, stop=False)
                                nc.tensor.matmul(o_ps[:, qb * 128:(qb + 1) * 128],
                                                 vhalo[:, h * DH:(h + 1) * DH], ah[:],
                                                 start=False, stop=True)
                            # attn-LIF step t (vth=0.5 -> threshold 1.0 on u)
                            if u_o is None:
                                u = upool.tile([DH, NCH], BF16, name="uo", tag="uo")
                                nc.vector.tensor_copy(u[:], o_ps[:])
                            else:
                                g = gpool.tile([DH, NCH], BF16, name="go", tag="go")
                                nc.vector.tensor_scalar(g[:], u_o[:], VTH2A, 0.5,
                                                        ALU.is_lt, ALU.mult)
                                ug = gpool.tile([DH, NCH], BF16, name="ugo", tag="ugo")
                                nc.vector.tensor_tensor(ug[:], u_o[:], g[:], ALU.mult)
                                u = upool.tile([DH, NCH], BF16, name="uo", tag="uo")
                                nc.vector.tensor_tensor(u[:], ug[:], o_ps[:], ALU.add)
                            nc.vector.tensor_scalar(
                                oa[h][:, t * NCH:(t + 1) * NCH], u[:], VTH2A, None,
                                ALU.is_ge)
                            u_o = u

                    # halo captures for the NEXT chunk (emit after attention reads)
                    for h in range(NH):
                        nc.vector.tensor_copy(
                            kh_prev[h][0:96, :, :],
                            k_ext[h][0:96, :].rearrange("p (t n) -> p t n", t=T)
                            [:, :, NCH - 8:NCH])
                    for t in range(T):
                        nc.sync.dma_start(vh_prev[t][:], vt_t[t][1][120:128, :])

                    # ---- proj GEMM + LIF -> opT scratch ----
                    for i in range(CI6):
                        psp = qkv_ps.tile([128, T, NCH], F32, name="qkvps", tag="qkvps")
                        for hi in range(NH):
                            lhsT = wp_sb[0:DH, hi * C + i * 128:hi * C + (i + 1) * 128]
                            rhs3 = oa[hi][:, :].rearrange("p (t n) -> p t n", t=T)
                            for hf in range(2):
                                nc.tensor.matmul(
                                    psp[:, 2 * hf:2 * hf + 2, :], lhsT,
                                    rhs3[:, 2 * hf:2 * hf + 2, :],
                                    start=(hi == 0), stop=(hi == NH - 1))
                        opc = oppool.tile([128, T, NCH], BF16, name="opc", tag="opc")
                        _lif_head(nc, (upool, gpool),
                                  lambda t, psp=psp: psp[:, t, :],
                                  bp_sb[:, i:i + 1],
                                  lambda t, opc=opc: opc[:, t, :],
                                  VTH2)
                        nc.sync.dma_start(tok3(opT[i], c), opc[:])

                    if debug:
                        for h in range(NH):
                            nc.sync.dma_start(
                                dbg_q.rearrange("h d (t n) -> h d t n", t=T)
                                [h][:, :, c * NCH:(c + 1) * NCH],
                                q_ext[h][0:96, :].rearrange("p (t n) -> p t n", t=T))
                            nc.sync.dma_start(
                                dbg_k.rearrange("h d (t n) -> h d t n", t=T)
                                [h][:, :, c * NCH:(c + 1) * NCH],
                                k_ext[h][0:96, :].rearrange("p (t n) -> p t n", t=T))
                            nc.sync.dma_start(
                                dbg_oa.rearrange("h d (t n) -> h d t n", t=T)
                                [h][:, :, c * NCH:(c + 1) * NCH],
                                oa[h][:, :].rearrange("p (t n) -> p t n", t=T))
                        for t in range(T):
                            for qh in range(2):
                                nc.sync.dma_start(
                                    dbg_v[t * NSEQ + c * NCH + qh * 128:
                                          t * NSEQ + c * NCH + (qh + 1) * 128, :],
                                    vt_t[t][qh][:])

            pa.close()
            # ======================= PHASE B =======================
            with ExitStack() as pb:
                wpoolB = pb.enter_context(tc.tile_pool(name="wB", bufs=1))
                xbpool = pb.enter_context(tc.tile_pool(name="xB", bufs=1))
                hpool = pb.enter_context(tc.tile_pool(name="hB", bufs=1))
                ubpool = pb.enter_context(tc.tile_pool(name="uB", bufs=3))
                gbpool = pb.enter_context(tc.tile_pool(name="gB", bufs=2))
                obpool = pb.enter_context(tc.tile_pool(name="oB", bufs=2))
                b_ps = pb.enter_context(tc.tile_pool(name="bps", bufs=3, space="PSUM"))

                w1_sb = wpoolB.tile([128, CI6 * HD], BF16, name="w1", tag="w1")
                nc.sync.dma_start(w1_sb[:], w1_d[:])
                w2_sb = wpoolB.tile([128, M24 * C], BF16, name="w2", tag="w2")
                nc.sync.dma_start(w2_sb[:], w2_d[:])
                b1_sb = wpoolB.tile([128, M24], F32, name="b1", tag="b1")
                nc.sync.dma_start(b1_sb[:], b1_d[:])
                b2_sb = wpoolB.tile([128, CI6], F32, name="b2", tag="b2")
                nc.sync.dma_start(b2_sb[:], b2_d[:])

                for c in range(NCHUNK):
                    xb, opb, x2 = [], [], []
                    for i in range(CI6):
                        xi = xbpool.tile([128, T, NCH], BF16, name=f"xb{i}", tag=f"xb{i}")
                        nc.sync.dma_start(xi[:], tok3(xT_bf[i], c))
                        xb.append(xi)
                        oi = xbpool.tile([128, T, NCH], BF16, name=f"ob{i}", tag=f"ob{i}")
                        nc.sync.dma_start(oi[:], tok3(opT[i], c))
                        opb.append(oi)
                        x2i = xbpool.tile([128, T, NCH], BF16, name=f"x2{i}", tag=f"x2{i}")
                        nc.vector.tensor_tensor(x2i[:], xi[:], oi[:], ALU.add)
                        x2.append(x2i)

                    h_tiles = []
                    for m in range(M24):
                        ps1 = b_ps.tile([128, T, NCH], F32, name="bps", tag="bps")
                        for ci in range(CI6):
                            lhsT = w1_sb[:, ci * HD + m * 128:ci * HD + (m + 1) * 128]
                            for hf in range(2):
                                nc.tensor.matmul(
                                    ps1[:, 2 * hf:2 * hf + 2, :], lhsT,
                                    x2[ci][:, 2 * hf:2 * hf + 2, :],
                                    start=(ci == 0), stop=(ci == CI6 - 1))
                        hm = hpool.tile([128, T, NCH], BF16, name=f"h{m}", tag=f"h{m}")
                        _lif_head(nc, (ubpool, gbpool),
                                  lambda t, ps1=ps1: ps1[:, t, :],
                                  b1_sb[:, m:m + 1],
                                  lambda t, hm=hm: hm[:, t, :],
                                  VTH2)
                        h_tiles.append(hm)
                        if debug:
                            nc.sync.dma_start(
                                dbg_h.rearrange("m p (t n) -> m p t n", t=T)
                                [m][:, :, c * NCH:(c + 1) * NCH], hm[:])

                    for i in range(CI6):
                        ps2 = b_ps.tile([128, T, NCH], F32, name="bps", tag="bps")
                        for k in range(M24):
                            lhsT = w2_sb[:, k * C + i * 128:k * C + (i + 1) * 128]
                            for hf in range(2):
                                nc.tensor.matmul(
                                    ps2[:, 2 * hf:2 * hf + 2, :], lhsT,
                                    h_tiles[k][:, 2 * hf:2 * hf + 2, :],
                                    start=(k == 0), stop=(k == M24 - 1))
                        msp = obpool.tile([128, T, NCH], BF16, name="msp", tag="msp")
                        _lif_head(nc, (ubpool, gbpool),
                                  lambda t, ps2=ps2: ps2[:, t, :],
                                  b2_sb[:, i:i + 1],
                                  lambda t, msp=msp: msp[:, t, :],
                                  VTH2)
                        xf = obpool.tile([128, T, NCH], F32, name="xf", tag="xf")
                        nc.sync.dma_start(xf[:], tok3(xT_f32[i], c))
                        nc.vector.tensor_tensor(xf[:], xf[:], opb[i][:], ALU.add)
                        nc.vector.tensor_tensor(xf[:], xf[:], msp[:], ALU.add)
                        nc.sync.dma_start(tok3(outT[i], c), xf[:])

    nc.compile()
    return nc


# ---------------- host-side preparation ----------------

def _fold(w, s):
    return (w * s[:, None]).astype(np.float32)


def _prep_shared(qw, qb, qs, qt, kw, kb, ks, kt, vw, vb, vs, vt,
                 pw, pb, ps, pt, f1w, f1b, f1s, f1t, f2w, f2b, f2s, f2t):
    """weights/biases/patterns shared by all cores"""
    out = {}
    # q/k: lhsT tiles [128, (ci,h,dh)] : w'[96h+m, 128ci+p]
    for name, w, bb, ss, tt in (("q", qw, qb, qs, qt), ("k", kw, kb, ks, kt)):
        wf = _fold(w, ss)                      # [C, C] = [out, in]
        arr = np.empty((128, CI6 * NH * DH), dtype=BF)
        for ci in range(CI6):
            for h in range(NH):
                blk = wf[h * DH:(h + 1) * DH, ci * 128:(ci + 1) * 128]  # [96,128]
                arr[:, (ci * NH + h) * DH:(ci * NH + h + 1) * DH] = blk.T.astype(BF)
        out["w" + name] = arr
        bias = (bb * ss + tt).astype(np.float32)          # [C]
        out["b" + name] = np.ascontiguousarray(
            bias.reshape(NH, DH).T)                       # [96, 8]
    # v: moving tiles [128, ci*768+o] = w'[o, 128ci+p]
    wf = _fold(vw, vs)
    arr = np.empty((128, CI6 * C), dtype=BF)
    for ci in range(CI6):
        arr[:, ci * C:(ci + 1) * C] = wf[:, ci * 128:(ci + 1) * 128].T.astype(BF)
    out["wv"] = arr
    bv = (vb * vs + vt).astype(np.float32)
    out["bvf"] = np.tile(bv[None, :], (128, 1)).astype(BF)
    # proj: lhsT [96, hi*768+o] = w'[o, 96hi+p]
    wf = _fold(pw, ps)
    arr = np.empty((DH, NH * C), dtype=BF)
    for hi in range(NH):
        arr[:, hi * C:(hi + 1) * C] = wf[:, hi * DH:(hi + 1) * DH].T.astype(BF)
    out["wp"] = arr
    bpv = (pb * ps + pt).astype(np.float32)
    out["bp"] = np.ascontiguousarray(bpv.reshape(CI6, 128).T)     # [128, 6]
    # f1: [128, ci*3072+o] = w'[o, 128ci+p]
    wf = _fold(f1w, f1s)
    arr = np.empty((128, CI6 * HD), dtype=BF)
    for ci in range(CI6):
        arr[:, ci * HD:(ci + 1) * HD] = wf[:, ci * 128:(ci + 1) * 128].T.astype(BF)
    out["w1"] = arr
    b1v = (f1b * f1s + f1t).astype(np.float32)
    out["b1"] = np.ascontiguousarray(b1v.reshape(M24, 128).T)     # [128, 24]
    # f2: [128, k*768+o] = w'[o, 128k+p]
    wf = _fold(f2w, f2s)
    arr = np.empty((128, M24 * C), dtype=BF)
    for k in range(M24):
        arr[:, k * C:(k + 1) * C] = wf[:, k * 128:(k + 1) * 128].T.astype(BF)
    out["w2"] = arr
    b2v = (f2b * f2s + f2t).astype(np.float32)
    out["b2"] = np.ascontiguousarray(b2v.reshape(CI6, 128).T)     # [128, 6]

    # attention mask / ext patterns
    cols = NCH * T
    qp = np.zeros((16, cols), dtype=BF)
    kp = np.zeros((16, cols), dtype=np.float32)
    for col in range(cols):
        j = col % NCH
        jm = j % 128
        g = jm // W
        qp[g, col] = 1.0
        jwin = jm + W
        for gg in range(16):
            kp[gg, col] = 0.0 if (W * gg <= jwin < W * gg + 2 * W) else NEG
    out["qext_pat"] = qp
    out["kext_pat"] = kp.astype(BF)
    khp = np.full((16, T * W), NEG, dtype=np.float32)
    khp[0, :] = 0.0                       # lookback valid only for group 0
    out["khalo_pat"] = khp.astype(BF)
    out["khalo_first"] = np.full((16, T * W), NEG, dtype=BF)
    out["ident"] = np.eye(128, dtype=BF)
    return out


def prep_in_maps(inputs):
    x = np.asarray(inputs["x"], dtype=np.float32)
    shared = _prep_shared(**{k: np.asarray(v, np.float32)
                             for k, v in inputs.items() if k != "x"})
    in_maps = []
    for b in range(B):
        xt = x[:, b].reshape(TOK, C)                 # [4096, 768]
        xT = np.ascontiguousarray(xt.T).reshape(CI6, 128, TOK)
        m = dict(shared)
        m["xT_f32"] = xT
        m["xT_bf"] = xT.astype(BF)
        in_maps.append(m)
    return in_maps


_NC_CACHE = {}


def get_nc(debug=False):
    if debug not in _NC_CACHE:
        _NC_CACHE[debug] = build_nc(debug)
    return _NC_CACHE[debug]


def assemble_output(results):
    out = np.empty((T, B, NSEQ, C), dtype=np.float32)
    for b in range(B):
        oT = results[b]["outT"].reshape(C, TOK)       # [768, 4096]
        out[:, b] = oT.T.reshape(T, NSEQ, C)
    return out


def kernel(**inputs):
    nc = get_nc(debug=False)
    in_maps = prep_in_maps(inputs)
    res = run_bass_kernel_spmd(nc, in_maps, list(range(B)))
    return assemble_output(res.results)


# revision 10
# speedup vs baseline: 67.2255x; 67.2255x over previous
"""Trainium2 Bass kernel for nn_Block_59433757442280 (spiking-NN local-attention block).

Sharding: data-parallel over B=8 (one batch element per NeuronCore), no collectives.
On-device layout: activations transposed [C, tok] with tok = t*1024 + n.
All GEMMs in bf16 (validated: reproduces the fp32 reference exactly on these inputs,
the LIF threshold margins downstream are structural); LIF membrane state in bf16,
softmax in fp32/bf16. The local-attention mask is folded into the sim matmul via
16 extra one-hot contraction rows.
"""

import sys

for _p in ("/opt/trn_rl_repo",):
    if _p not in sys.path:
        sys.path.insert(0, _p)

import numpy as np
import ml_dtypes

import concourse.bass as bass
import concourse.tile as tile
from concourse import mybir, bacc
from concourse.bass_utils import run_bass_kernel_spmd

F32 = mybir.dt.float32
BF16 = mybir.dt.bfloat16
AF = mybir.ActivationFunctionType
ALU = mybir.AluOpType
BF = ml_dtypes.bfloat16

# problem constants
T, B, NSEQ, C, HD = 4, 8, 1024, 768, 3072
NH, DH, W = 8, 96, 8
TOK = T * NSEQ                      # 4096 tokens per core
SCALE = float(DH) ** -0.5
NEG = -30000.0                      # mask offset (exp(scale*NEG) == 0.0 in fp32)
NCH = 256                           # phase-A/B chunk size along n
NCHUNK = NSEQ // NCH                # 4 chunks
CI6 = C // 128                      # 6 contraction tiles of 128
M24 = HD // 128                     # 24 f1 output tiles
VTH2 = 2.0                          # 2*vth for qkv/proj/mlp LIF
VTH2A = 1.0                         # 2*vth for attn lif (vth=0.5)


def _lif_head(nc, pools, psum_ap_of_t, bias_ap, dst_of_t, vth2, nt=T):
    """Emit LIF over t for one tile group.

    psum_ap_of_t(t) -> [P, n] fp32 PSUM AP of the GEMM output at step t
    bias_ap        -> [P, 1] f32 SBUF AP (per-partition bias) or None
    dst_of_t(t)    -> [P, n] bf16 SBUF AP to write spikes into
    u state: u_{t+1} = u_t * (0.5*[u_t < vth2]) + y_{t+1};  s_t = [u_t >= vth2]
    """
    upool, gpool = pools
    u_prev = None
    for t in range(nt):
        y = psum_ap_of_t(t)
        p, n = y.shape[0], y.shape[-1]
        if u_prev is None:
            u = upool.tile([p, n], BF16, name="u", tag="u")
            if bias_ap is not None:
                nc.scalar.activation(u[:], y, AF.Identity, bias=bias_ap)
            else:
                nc.vector.tensor_copy(u[:], y)
        else:
            g = gpool.tile([p, n], BF16, name="g", tag="g")
            nc.vector.tensor_scalar(g[:], u_prev[:], vth2, 0.5, ALU.is_lt, ALU.mult)
            ug = gpool.tile([p, n], BF16, name="ug", tag="ug")
            nc.gpsimd.tensor_tensor(ug[:], u_prev[:], g[:], ALU.mult)
            if bias_ap is not None:
                yb = gpool.tile([p, n], BF16, name="yb", tag="yb")
                nc.scalar.activation(yb[:], y, AF.Identity, bias=bias_ap)
                u = upool.tile([p, n], BF16, name="u", tag="u")
                nc.vector.tensor_tensor(u[:], ug[:], yb[:], ALU.add)
            else:
                u = upool.tile([p, n], BF16, name="u", tag="u")
                nc.vector.tensor_tensor(u[:], ug[:], y, ALU.add)
        nc.vector.tensor_scalar(dst_of_t(t), u[:], vth2, None, ALU.is_ge)
        u_prev = u


def build_nc(debug=False):
    nc = bacc.Bacc(None, target_bir_lowering=False, debug=False)

    # ---- DRAM declarations (per core) ----
    xT_bf = nc.dram_tensor("xT_bf", [CI6, 128, TOK], BF16, kind="ExternalInput")
    xT_f32 = nc.dram_tensor("xT_f32", [CI6, 128, TOK], F32, kind="ExternalInput")
    wq_d = nc.dram_tensor("wq", [128, CI6 * C], BF16, kind="ExternalInput")
    wk_d = nc.dram_tensor("wk", [128, CI6 * C], BF16, kind="ExternalInput")
    wv_d = nc.dram_tensor("wv", [128, CI6 * C], BF16, kind="ExternalInput")
    wp_d = nc.dram_tensor("wp", [128, CI6 * C], BF16, kind="ExternalInput")
    w1_d = nc.dram_tensor("w1", [128, CI6 * HD], BF16, kind="ExternalInput")
    w2_d = nc.dram_tensor("w2", [128, M24 * C], BF16, kind="ExternalInput")
    bq_d = nc.dram_tensor("bq", [128, CI6], F32, kind="ExternalInput")
    bk_d = nc.dram_tensor("bk", [128, CI6], F32, kind="ExternalInput")
    bvf_d = nc.dram_tensor("bvf", [128, C], BF16, kind="ExternalInput")
    bp_d = nc.dram_tensor("bp", [128, CI6], F32, kind="ExternalInput")
    b1_d = nc.dram_tensor("b1", [128, M24], F32, kind="ExternalInput")
    b2_d = nc.dram_tensor("b2", [128, CI6], F32, kind="ExternalInput")
    qext_d = nc.dram_tensor("qext_pat", [16, NCH * T], BF16, kind="ExternalInput")
    kext_d = nc.dram_tensor("kext_pat", [16, NCH * T], BF16, kind="ExternalInput")
    khp_d = nc.dram_tensor("khalo_pat", [16, T * W], BF16, kind="ExternalInput")
    khf_d = nc.dram_tensor("khalo_first", [16, T * W], BF16, kind="ExternalInput")
    id_d = nc.dram_tensor("ident", [128, 128], BF16, kind="ExternalInput")

    opT = nc.dram_tensor("opT", [CI6, 128, TOK], BF16,
                         kind="ExternalOutput" if debug else "Internal")
    outT = nc.dram_tensor("outT", [CI6, 128, TOK], F32, kind="ExternalOutput")
    if debug:
        dbg_q = nc.dram_tensor("dbg_q", [NH, DH, TOK], BF16, kind="ExternalOutput")
        dbg_k = nc.dram_tensor("dbg_k", [NH, DH, TOK], BF16, kind="ExternalOutput")
        dbg_v = nc.dram_tensor("dbg_v", [TOK, C], BF16, kind="ExternalOutput")
        dbg_oa = nc.dram_tensor("dbg_oa", [NH, DH, TOK], BF16, kind="ExternalOutput")
        dbg_h = nc.dram_tensor("dbg_h", [M24, 128, TOK], BF16, kind="ExternalOutput")

    def tok3(dram_i, c):
        """chunk AP [128, T, NCH] of dram tensor slice i at chunk c"""
        return dram_i.rearrange("p (t n) -> p t n", t=T)[:, :, c * NCH:(c + 1) * NCH]

    with tile.TileContext(nc) as tc:
        from contextlib import ExitStack
        with ExitStack() as top:
            # ======================= PHASE A =======================
            pa = top.enter_context(ExitStack())
            cpool = pa.enter_context(tc.tile_pool(name="const", bufs=1))
            # persistent attention tiles
            perspool = pa.enter_context(tc.tile_pool(name="pers", bufs=1))

            ident = cpool.tile([128, 128], BF16, name="ident", tag="ident")
            nc.sync.dma_start(ident[:], id_d[:])
            bq_sb = cpool.tile([128, CI6], F32, name="bq", tag="bq")
            nc.sync.dma_start(bq_sb[:], bq_d[:])
            bk_sb = cpool.tile([128, CI6], F32, name="bk", tag="bk")
            nc.sync.dma_start(bk_sb[:], bk_d[:])
            bvf_sb = cpool.tile([128, C], BF16, name="bvf", tag="bvf")
            nc.sync.dma_start(bvf_sb[:], bvf_d[:])
            bp_sb = cpool.tile([128, CI6], F32, name="bp", tag="bp")
            nc.sync.dma_start(bp_sb[:], bp_d[:])

            q_ext = [perspool.tile([112, T * NCH], BF16, name=f"qx{h}", tag=f"qx{h}") for h in range(NH)]
            k_ext = [perspool.tile([112, T * NCH], BF16, name=f"kx{h}", tag=f"kx{h}") for h in range(NH)]
            kh_cur = [perspool.tile([112, T, W], BF16, name=f"khc{h}", tag=f"khc{h}") for h in range(NH)]
            kh_prev = [perspool.tile([112, T, W], BF16, name=f"khp{h}", tag=f"khp{h}") for h in range(NH)]
            kh_first = [perspool.tile([112, T, W], BF16, name=f"khf{h}", tag=f"khf{h}") for h in range(NH)]
            vt_t = [[perspool.tile([128, C], BF16, name=f"vt{t}_{qh}", tag=f"vt{t}_{qh}") for qh in range(2)]
                    for t in range(T)]
            vh_cur = [perspool.tile([W, C], BF16, name=f"vhc{t}", tag=f"vhc{t}") for t in range(T)]
            vh_prev = [perspool.tile([W, C], BF16, name=f"vhp{t}", tag=f"vhp{t}") for t in range(T)]
            oa = [perspool.tile([DH, T * NCH], BF16, name=f"oa{h}", tag=f"oa{h}") for h in range(NH)]

            # init ext rows / halos
            for h in range(NH):
                nc.sync.dma_start(q_ext[h][96:112, :], qext_d[:])
                nc.sync.dma_start(k_ext[h][96:112, :], kext_d[:])
                nc.sync.dma_start(
                    kh_cur[h][96:112, :, :], khp_d.rearrange("g (t w) -> g t w", t=T))
                nc.sync.dma_start(
                    kh_prev[h][96:112, :, :], khp_d.rearrange("g (t w) -> g t w", t=T))
                nc.sync.dma_start(
                    kh_first[h][96:112, :, :], khf_d.rearrange("g (t w) -> g t w", t=T))
                nc.vector.memset(kh_prev[h][0:96, :, :], 0.0)
                nc.vector.memset(kh_first[h][0:96, :, :], 0.0)
            for t in range(T):
                nc.vector.memset(vh_prev[t][:], 0.0)

            if True:
                wpoolA = pa.enter_context(tc.tile_pool(name="wA", bufs=1))
                xpool = pa.enter_context(tc.tile_pool(name="xA", bufs=2))
                upool = pa.enter_context(tc.tile_pool(name="uA", bufs=3))
                gpool = pa.enter_context(tc.tile_pool(name="gA", bufs=2))
                apool = pa.enter_context(tc.tile_pool(name="attn", bufs=3))
                oppool = pa.enter_context(tc.tile_pool(name="op", bufs=2))
                spkpool = pa.enter_context(tc.tile_pool(name="spk", bufs=1))
                qkv_ps = pa.enter_context(
                    tc.tile_pool(name="qkvps", bufs=2, space="PSUM"))
                sm_ps = pa.enter_context(
                    tc.tile_pool(name="smps", bufs=4, space="PSUM"))

                wq_sb = wpoolA.tile([128, CI6 * C], BF16, name="wq", tag="wq")
                nc.scalar.dma_start(wq_sb[:], wq_d[:])
                wk_sb = wpoolA.tile([128, CI6 * C], BF16, name="wk", tag="wk")
                nc.gpsimd.dma_start(wk_sb[:], wk_d[:])
                wv_sb = wpoolA.tile([128, CI6 * C], BF16, name="wv", tag="wv")
                nc.sync.dma_start(wv_sb[:], wv_d[:])
                wp_sb = wpoolA.tile([128, CI6 * C], BF16, name="wp", tag="wp")
                nc.scalar.dma_start(wp_sb[:], wp_d[:])

                for c in range(NCHUNK):
                    first_chunk = (c == 0)
                    # ---- load x chunk ----
                    xc = []
                    for i in range(CI6):
                        xi = xpool.tile([128, T, NCH], BF16, name=f"xc{i}", tag=f"xc{i}")
                        nc.sync.dma_start(xi[:], tok3(xT_bf[i], c))
                        xc.append(xi)

                    # ---- q, k GEMM (M=128) + LIF + repack -> q_ext/k_ext rows 0:96 ----
                    for w_sb, b_sb, ext, snm in ((wq_sb, bq_sb, q_ext, "qs"),
                                                 (wk_sb, bk_sb, k_ext, "ks")):
                        s_tmp = []
                        for i in range(CI6):
                            ps = qkv_ps.tile([128, T, NCH], F32, name="qkvps", tag="qkvps")
                            for ci in range(CI6):
                                lhsT = w_sb[:, ci * C + i * 128:ci * C + (i + 1) * 128]
                                for hf in range(2):
                                    nc.tensor.matmul(
                                        ps[:, 2 * hf:2 * hf + 2, :], lhsT,
                                        xc[ci][:, 2 * hf:2 * hf + 2, :],
                                        start=(ci == 0), stop=(ci == CI6 - 1))
                            st = spkpool.tile([128, T * NCH], BF16, name=f"{snm}{i}",
                                              tag=f"{snm}{i}")
                            _lif_head(
                                nc, (upool, gpool),
                                lambda t, ps=ps: ps[:, t, :],
                                b_sb[:, i:i + 1],
                                lambda t, st=st: st[:, t * NCH:(t + 1) * NCH],
                                VTH2)
                            s_tmp.append(st)
                        for h in range(NH):
                            cst = h * DH
                            i0, r0 = cst // 128, cst % 128
                            l0 = min(128 - r0, DH)
                            nc.sync.dma_start(ext[h][0:l0, :], s_tmp[i0][r0:r0 + l0, :])
                            if l0 < DH:
                                nc.sync.dma_start(ext[h][l0:DH, :],
                                                  s_tmp[i0 + 1][0:DH - l0, :])

                    # ---- v GEMM (x-stationary -> v.T layout) + LIF ----
                    for qh in range(2):
                        psv_of_t = []
                        for t in range(T):
                            psv = qkv_ps.tile([128, C], F32, name="qkvps", tag="qkvps")
                            for ci in range(CI6):
                                stat = xc[ci][:, t, qh * 128:(qh + 1) * 128]
                                nc.tensor.matmul(psv[:, 0:512], stat,
                                                 wv_sb[:, ci * C:ci * C + 512],
                                                 start=(ci == 0), stop=(ci == CI6 - 1))
                                nc.tensor.matmul(psv[:, 512:C], stat,
                                                 wv_sb[:, ci * C + 512:(ci + 1) * C],
                                                 start=(ci == 0), stop=(ci == CI6 - 1))
                            psv_of_t.append(psv)
                        # LIF over t in v.T layout with full-width bias
                        u_prev = None
                        for t in range(T):
                            y = psv_of_t[t]
                            if u_prev is None:
                                u = upool.tile([128, C], BF16, name="uv", tag="uv")
                                nc.vector.tensor_tensor(u[:], y[:], bvf_sb[:], ALU.add)
                            else:
                                g = gpool.tile([128, C], BF16, name="gv", tag="gv")
                                nc.vector.tensor_scalar(g[:], u_prev[:], VTH2, 0.5,
                                                        ALU.is_lt, ALU.mult)
                                ug = gpool.tile([128, C], BF16, name="ugv", tag="ugv")
                                nc.gpsimd.tensor_tensor(ug[:], u_prev[:], g[:], ALU.mult)
                                ub = gpool.tile([128, C], BF16, name="ubv", tag="ubv")
                                nc.gpsimd.tensor_tensor(ub[:], ug[:], bvf_sb[:], ALU.add)
                                u = upool.tile([128, C], BF16, name="uv", tag="uv")
                                nc.vector.tensor_tensor(u[:], ub[:], y[:], ALU.add)
                            nc.vector.tensor_scalar(vt_t[t][qh][:], u[:], VTH2, None,
                                                    ALU.is_ge)
                            u_prev = u

                    # halo captures needed within this chunk (qb=1 halos)
                    for h in range(NH):
                        nc.vector.tensor_copy(
                            kh_cur[h][0:96, :, :],
                            k_ext[h][0:96, :].rearrange("p (t n) -> p t n", t=T)
                            [:, :, 120:128])
                    for t in range(T):
                        nc.sync.dma_start(vh_cur[t][:], vt_t[t][0][120:128, :])

                    # ---- attention + attn-LIF -> oa ----
                    for h in range(NH):
                        u_o = None
                        for t in range(T):
                            o_ps = sm_ps.tile([DH, NCH], F32, name="attnsm", tag="attnsm")
                            for qb in range(2):
                                qc = t * NCH + qb * 128
                                sim = sm_ps.tile([128, 136], F32, name="attnsm", tag="attnsm")
                                nc.tensor.matmul(
                                    sim[:, 0:128], q_ext[h][0:112, qc:qc + 128],
                                    k_ext[h][0:112, qc:qc + 128], start=True, stop=True)
                                halo = (kh_first[h] if (first_chunk and qb == 0)
                                        else kh_prev[h] if qb == 0 else kh_cur[h])
                                nc.tensor.matmul(
                                    sim[:, 128:136], q_ext[h][0:112, qc:qc + 128],
                                    halo[0:112, t, :], start=True, stop=True)
                                attn = apool.tile([128, 136], BF16, name="attn", tag="attn")
                                rs = apool.tile([128, 1], F32, name="rs", tag="rs")
                                nc.scalar.activation(attn[:], sim[:], AF.Exp,
                                                     scale=SCALE, accum_out=rs[:])
                                rc = apool.tile([128, 1], F32, name="rc", tag="rc")
                                nc.vector.reciprocal(rc[:], rs[:])
                                attn_n = apool.tile([128, 136], BF16, name="attnn", tag="attnn")
                                nc.vector.tensor_scalar(attn_n[:], attn[:], rc[:], None,
                                                        ALU.mult)
                                tpm = sm_ps.tile([128, 128], BF16, name="attnsm", tag="attnsm")
                                nc.tensor.transpose(tpm[:], attn_n[:, 0:128], ident[:])
                                tph = sm_ps.tile([8, 128], BF16, name="attnsm", tag="attnsm")
                                nc.tensor.transpose(tph[:], attn_n[:, 128:136], ident[:])
                                am = apool.tile([128, 128], BF16, name="am", tag="am")
                                nc.scalar.copy(am[:], tpm[:])
                                ah = apool.tile([8, 128], BF16, name="ah", tag="ah")
                                nc.vector.tensor_copy(ah[:], tph[:])
                                vmain = vt_t[t][qb][:, h * DH:(h + 1) * DH]
                                vhalo = (vh_prev[t] if qb == 0 else vh_cur[t])
                                nc.tensor.matmul(o_ps[:, qb * 128:(qb + 1) * 128],
                                                 vmain, am[:], start=True, stop=False)
                                nc.tensor.matmul(o_ps[:, qb * 128:(qb + 1) * 128],
                                                 vhalo[:, h * DH:(h + 1) * DH], ah[:],
                                                 start=False, stop=True)
                            # attn-LIF step t (vth=0.5 -> threshold 1.0 on u)
                            if u_o is None:
                                u = upool.tile([DH, NCH], BF16, name="uo", tag="uo")
                                nc.scalar.copy(u[:], o_ps[:])
                            else:
                                g = gpool.tile([DH, NCH], BF16, name="go", tag="go")
                                nc.vector.tensor_scalar(g[:], u_o[:], VTH2A, 0.5,
                                                        ALU.is_lt, ALU.mult)
                                ug = gpool.tile([DH, NCH], BF16, name="ugo", tag="ugo")
                                nc.gpsimd.tensor_tensor(ug[:], u_o[:], g[:], ALU.mult)
                                u = upool.tile([DH, NCH], BF16, name="uo", tag="uo")
                                nc.vector.tensor_tensor(u[:], ug[:], o_ps[:], ALU.add)
                            nc.vector.tensor_scalar(
                                oa[h][:, t * NCH:(t + 1) * NCH], u[:], VTH2A, None,
                                ALU.is_ge)
                            u_o = u

                    # halo captures for the NEXT chunk (emit after attention reads)
                    for h in range(NH):
                        nc.vector.tensor_copy(
                            kh_prev[h][0:96, :, :],
                            k_ext[h][0:96, :].rearrange("p (t n) -> p t n", t=T)
                            [:, :, NCH - 8:NCH])
                    for t in range(T):
                        nc.sync.dma_start(vh_prev[t][:], vt_t[t][1][120:128, :])

                    # ---- oa repack to 128-tiles, proj GEMM (K=128) + LIF ----
                    oa128 = []
                    for i in range(CI6):
                        ot = spkpool.tile([128, T * NCH], BF16, name=f"oa128_{i}",
                                          tag=f"oa128_{i}")
                        oa128.append(ot)
                    for h in range(NH):
                        cst = h * DH
                        i0, r0 = cst // 128, cst % 128
                        l0 = min(128 - r0, DH)
                        nc.sync.dma_start(oa128[i0][r0:r0 + l0, :], oa[h][0:l0, :])
                        if l0 < DH:
                            nc.sync.dma_start(oa128[i0 + 1][0:DH - l0, :],
                                              oa[h][l0:DH, :])
                    for i in range(CI6):
                        psp = qkv_ps.tile([128, T, NCH], F32, name="qkvps", tag="qkvps")
                        for ci in range(CI6):
                            lhsT = wp_sb[:, ci * C + i * 128:ci * C + (i + 1) * 128]
                            rhs3 = oa128[ci][:, :].rearrange("p (t n) -> p t n", t=T)
                            for hf in range(2):
                                nc.tensor.matmul(
                                    psp[:, 2 * hf:2 * hf + 2, :], lhsT,
                                    rhs3[:, 2 * hf:2 * hf + 2, :],
                                    start=(ci == 0), stop=(ci == CI6 - 1))
                        opc = oppool.tile([128, T, NCH], BF16, name="opc", tag="opc")
                        _lif_head(nc, (upool, gpool),
                                  lambda t, psp=psp: psp[:, t, :],
                                  bp_sb[:, i:i + 1],
                                  lambda t, opc=opc: opc[:, t, :],
                                  VTH2)
                        nc.sync.dma_start(tok3(opT[i], c), opc[:])

                    if debug:
                        for h in range(NH):
                            nc.sync.dma_start(
                                dbg_q.rearrange("h d (t n) -> h d t n", t=T)
                                [h][:, :, c * NCH:(c + 1) * NCH],
                                q_ext[h][0:96, :].rearrange("p (t n) -> p t n", t=T))
                            nc.sync.dma_start(
                                dbg_k.rearrange("h d (t n) -> h d t n", t=T)
                                [h][:, :, c * NCH:(c + 1) * NCH],
                                k_ext[h][0:96, :].rearrange("p (t n) -> p t n", t=T))
                            nc.sync.dma_start(
                                dbg_oa.rearrange("h d (t n) -> h d t n", t=T)
                                [h][:, :, c * NCH:(c + 1) * NCH],
                                oa[h][:, :].rearrange("p (t n) -> p t n", t=T))
                        for t in range(T):
                            for qh in range(2):
                                nc.sync.dma_start(
                                    dbg_v[t * NSEQ + c * NCH + qh * 128:
                                          t * NSEQ + c * NCH + (qh + 1) * 128, :],
                                    vt_t[t][qh][:])

            pa.close()
            # ======================= PHASE B =======================
            with ExitStack() as pb:
                wpoolB = pb.enter_context(tc.tile_pool(name="wB", bufs=1))
                xbpool = pb.enter_context(tc.tile_pool(name="xB", bufs=1))
                hpool = pb.enter_context(tc.tile_pool(name="hB", bufs=1))
                ubpool = pb.enter_context(tc.tile_pool(name="uB", bufs=3))
                gbpool = pb.enter_context(tc.tile_pool(name="gB", bufs=2))
                obpool = pb.enter_context(tc.tile_pool(name="oB", bufs=2))
                b_ps = pb.enter_context(tc.tile_pool(name="bps", bufs=4, space="PSUM"))

                w1_sb = wpoolB.tile([128, CI6 * HD], BF16, name="w1", tag="w1")
                nc.scalar.dma_start(w1_sb[:, 0:CI6 * HD // 2], w1_d[:, 0:CI6 * HD // 2])
                nc.sync.dma_start(w1_sb[:, CI6 * HD // 2:], w1_d[:, CI6 * HD // 2:])
                w2_sb = wpoolB.tile([128, M24 * C], BF16, name="w2", tag="w2")
                nc.gpsimd.dma_start(w2_sb[:, 0:M24 * C // 2], w2_d[:, 0:M24 * C // 2])
                nc.gpsimd.dma_start(w2_sb[:, M24 * C // 2:], w2_d[:, M24 * C // 2:])
                b1_sb = wpoolB.tile([128, M24], F32, name="b1", tag="b1")
                nc.sync.dma_start(b1_sb[:], b1_d[:])
                b2_sb = wpoolB.tile([128, CI6], F32, name="b2", tag="b2")
                nc.sync.dma_start(b2_sb[:], b2_d[:])

                for c in range(NCHUNK):
                    xb, opb, x2 = [], [], []
                    for i in range(CI6):
                        xi = xbpool.tile([128, T, NCH], BF16, name=f"xb{i}", tag=f"xb{i}")
                        nc.sync.dma_start(xi[:], tok3(xT_bf[i], c))
                        xb.append(xi)
                        oi = xbpool.tile([128, T, NCH], BF16, name=f"ob{i}", tag=f"ob{i}")
                        nc.sync.dma_start(oi[:], tok3(opT[i], c))
                        opb.append(oi)
                        x2i = xbpool.tile([128, T, NCH], BF16, name=f"x2{i}", tag=f"x2{i}")
                        nc.gpsimd.tensor_tensor(x2i[:], xi[:], oi[:], ALU.add)
                        x2.append(x2i)

                    h_tiles = []
                    for m in range(M24):
                        ps1 = b_ps.tile([128, T, NCH], F32, name="bps", tag="bps")
                        for ci in range(CI6):
                            lhsT = w1_sb[:, ci * HD + m * 128:ci * HD + (m + 1) * 128]
                            for hf in range(2):
                                nc.tensor.matmul(
                                    ps1[:, 2 * hf:2 * hf + 2, :], lhsT,
                                    x2[ci][:, 2 * hf:2 * hf + 2, :],
                                    start=(ci == 0), stop=(ci == CI6 - 1))
                        hm = hpool.tile([128, T, NCH], BF16, name=f"h{m}", tag=f"h{m}")
                        _lif_head(nc, (ubpool, gbpool),
                                  lambda t, ps1=ps1: ps1[:, t, :],
                                  b1_sb[:, m:m + 1],
                                  lambda t, hm=hm: hm[:, t, :],
                                  VTH2)
                        h_tiles.append(hm)
                        if debug:
                            nc.sync.dma_start(
                                dbg_h.rearrange("m p (t n) -> m p t n", t=T)
                                [m][:, :, c * NCH:(c + 1) * NCH], hm[:])

                    for i in range(CI6):
                        ps2 = b_ps.tile([128, T, NCH], F32, name="bps", tag="bps")
                        for k in range(M24):
                            lhsT = w2_sb[:, k * C + i * 128:k * C + (i + 1) * 128]
                            for hf in range(2):
                                nc.tensor.matmul(
                                    ps2[:, 2 * hf:2 * hf + 2, :], lhsT,
                                    h_tiles[k][:, 2 * hf:2 * hf + 2, :],
                                    start=(k == 0), stop=(k == M24 - 1))
                        msp = obpool.tile([128, T, NCH], BF16, name="msp", tag="msp")
                        _lif_head(nc, (ubpool, gbpool),
                                  lambda t, ps2=ps2: ps2[:, t, :],
                                  b2_sb[:, i:i + 1],
                                  lambda t, msp=msp: msp[:, t, :],
                                  VTH2)
                        xf = obpool.tile([128, T, NCH], F32, name="xf", tag="xf")
                        nc.sync.dma_start(xf[:], tok3(xT_f32[i], c))
                        nc.gpsimd.tensor_tensor(xf[:], xf[:], opb[i][:], ALU.add)
                        nc.gpsimd.tensor_tensor(xf[:], xf[:], msp[:], ALU.add)
                        nc.sync.dma_start(tok3(outT[i], c), xf[:])

    nc.compile()
    return nc


# ---------------- host-side preparation ----------------

def _fold(w, s):
    return (w * s[:, None]).astype(np.float32)


def _prep_shared(qw, qb, qs, qt, kw, kb, ks, kt, vw, vb, vs, vt,
                 pw, pb, ps, pt, f1w, f1b, f1s, f1t, f2w, f2b, f2s, f2t):
    """weights/biases/patterns shared by all cores"""
    out = {}
    # q/k: lhsT tiles [128, ci*768+o] = w'[o, 128ci+p]
    for name, w, bb, ss, tt in (("q", qw, qb, qs, qt), ("k", kw, kb, ks, kt)):
        wf = _fold(w, ss)                      # [C, C] = [out, in]
        arr = np.empty((128, CI6 * C), dtype=BF)
        for ci in range(CI6):
            arr[:, ci * C:(ci + 1) * C] = wf[:, ci * 128:(ci + 1) * 128].T.astype(BF)
        out["w" + name] = arr
        bias = (bb * ss + tt).astype(np.float32)          # [C]
        out["b" + name] = np.ascontiguousarray(bias.reshape(CI6, 128).T)  # [128, 6]
    # v: moving tiles [128, ci*768+o] = w'[o, 128ci+p]
    wf = _fold(vw, vs)
    arr = np.empty((128, CI6 * C), dtype=BF)
    for ci in range(CI6):
        arr[:, ci * C:(ci + 1) * C] = wf[:, ci * 128:(ci + 1) * 128].T.astype(BF)
    out["wv"] = arr
    bv = (vb * vs + vt).astype(np.float32)
    out["bvf"] = np.tile(bv[None, :], (128, 1)).astype(BF)
    # proj: lhsT [128, ci*768+o] = w'[o, 128ci+p]
    wf = _fold(pw, ps)
    arr = np.empty((128, CI6 * C), dtype=BF)
    for ci in range(CI6):
        arr[:, ci * C:(ci + 1) * C] = wf[:, ci * 128:(ci + 1) * 128].T.astype(BF)
    out["wp"] = arr
    bpv = (pb * ps + pt).astype(np.float32)
    out["bp"] = np.ascontiguousarray(bpv.reshape(CI6, 128).T)     # [128, 6]
    # f1: [128, ci*3072+o] = w'[o, 128ci+p]
    wf = _fold(f1w, f1s)
    arr = np.empty((128, CI6 * HD), dtype=BF)
    for ci in range(CI6):
        arr[:, ci * HD:(ci + 1) * HD] = wf[:, ci * 128:(ci + 1) * 128].T.astype(BF)
    out["w1"] = arr
    b1v = (f1b * f1s + f1t).astype(np.float32)
    out["b1"] = np.ascontiguousarray(b1v.reshape(M24, 128).T)     # [128, 24]
    # f2: [128, k*768+o] = w'[o, 128k+p]
    wf = _fold(f2w, f2s)
    arr = np.empty((128, M24 * C), dtype=BF)
    for k in range(M24):
        arr[:, k * C:(k + 1) * C] = wf[:, k * 128:(k + 1) * 128].T.astype(BF)
    out["w2"] = arr
    b2v = (f2b * f2s + f2t).astype(np.float32)
    out["b2"] = np.ascontiguousarray(b2v.reshape(CI6, 128).T)     # [128, 6]

    # attention mask / ext patterns
    cols = NCH * T
    qp = np.zeros((16, cols), dtype=BF)
    kp = np.zeros((16, cols), dtype=np.float32)
    for col in range(cols):
        j = col % NCH
        jm = j % 128
        g = jm // W
        qp[g, col] = 1.0
        jwin = jm + W
        for gg in range(16):
            kp[gg, col] = 0.0 if (W * gg <= jwin < W * gg + 2 * W) else NEG
    out["qext_pat"] = qp
    out["kext_pat"] = kp.astype(BF)
    khp = np.full((16, T * W), NEG, dtype=np.float32)
    khp[0, :] = 0.0                       # lookback valid only for group 0
    out["khalo_pat"] = khp.astype(BF)
    out["khalo_first"] = np.full((16, T * W), NEG, dtype=BF)
    out["ident"] = np.eye(128, dtype=BF)
    return out


def prep_in_maps(inputs):
    x = np.asarray(inputs["x"], dtype=np.float32)
    shared = _prep_shared(**{k: np.asarray(v, np.float32)
                             for k, v in inputs.items() if k != "x"})
    in_maps = []
    for b in range(B):
        xt = x[:, b].reshape(TOK, C)                 # [4096, 768]
        xT = np.ascontiguousarray(xt.T).reshape(CI6, 128, TOK)
        m = dict(shared)
        m["xT_f32"] = xT
        m["xT_bf"] = xT.astype(BF)
        in_maps.append(m)
    return in_maps


_NC_CACHE = {}


def get_nc(debug=False):
    if debug not in _NC_CACHE:
        _NC_CACHE[debug] = build_nc(debug)
    return _NC_CACHE[debug]


def assemble_output(results):
    out = np.empty((T, B, NSEQ, C), dtype=np.float32)
    for b in range(B):
        oT = results[b]["outT"].reshape(C, TOK)       # [768, 4096]
        out[:, b] = oT.T.reshape(T, NSEQ, C)
    return out


def kernel(**inputs):
    nc = get_nc(debug=False)
    in_maps = prep_in_maps(inputs)
    res = run_bass_kernel_spmd(nc, in_maps, list(range(B)))
    return assemble_output(res.results)
